# revision 23
# baseline (speedup 1.0000x reference)
"""Trainium2 Bass kernel for nn_CircuitModel (sequential Hebbian/Oja scan).

Math
----
Reference (per timestep t, rows i):
    y_t = sigmoid(W_t x_t)
    W_{t+1} = diag(1 + a*y_t^2) W_t + b*y_t x_t^T,   a = lr*theta1, b = lr*theta0

Reformulation: W_t = diag(exp(S_t)) V_t with
    S_t[i] = a * sum_{s<t} y_s[i]^2          (log1p(a y^2) ~= a y^2)
    V_{t+1} = V_t + (b y_t exp(-S_{t+1})) x_t^T
    z_t = exp(S_t) * (V_t . x_t),  y_t = sigmoid(z_t)

Blocked fixed-point solver over sequential super-blocks of BB = NSUB*128
steps; SWEEPS Jacobi sweeps per super-block (contraction ~1e-1..1e-2).
exp is linearized (1+S), valid since |S| <= |a|*T ~ 5e-3 (asserted).
Defaults (NSUB=4, SWEEPS=2, no trailing half-sweep) measure ~75us in the
TimelineSim cost model on TRN2 with max elementwise rel err ~7e-4 vs the
fp64 reference; KERNEL_HALF=1 gives ~87us at ~4.7e-4 if more margin is
ever needed. Gram/cumsum matmuls run in float32r (fp32 rounded to 11-bit
mantissa, 4x faster PE streaming); the Z0/V-update path stays full fp32.

Per sweep: S_incl for ALL sub-tiles via one triangular matmul (each sub-tile's
local time index lives on the same 128 partitions) + all-ones cross-sub
matmuls + QQ carry matmul; E_ex = (S_in + 1) - a*y^2 on DVE; coupling via
masked-Gram matmuls; sigmoid/square on ACT. X stays resident in SBUF (~8MB).

Sharding: row-parallel over n_output, 64 rows of W per core, X broadcast;
cores fully independent. Observed-row gather on host.

Sign convention: uh' = (S_incl - 1)*Y = -y*exp_lin(-S_incl); coupling and
V-update matmuls produce negated values, compensated by subtracts.
"""

import os
import numpy as np

import concourse.bacc as bacc
import concourse.bass as bass
import concourse.mybir as mybir
import concourse.tile as tile
from concourse.bass_utils import run_bass_kernel_spmd

F32 = mybir.dt.float32
F32R = mybir.dt.float32r
BF16 = mybir.dt.bfloat16


def _f(ap):
    """Read a float32r AP as plain fp32 (f32r bits are valid fp32 values)."""
    return ap.bitcast(F32)

ALU = mybir.AluOpType
ACTF = mybir.ActivationFunctionType

N_IN = 512
N_OUT = 512
T = 2048
B = 128
MROWS = N_OUT // 8  # 64 rows per core
NCORES = 8
SWEEPS = int(os.environ.get("KERNEL_SWEEPS", "2"))
Y2ACT = bool(int(os.environ.get("KERNEL_Y2ACT", "0")))
Z0ACT = bool(int(os.environ.get("KERNEL_Z0ACT", "1")))
EEXPE = bool(int(os.environ.get("KERNEL_EEXPE", "0")))
GBUFS = int(os.environ.get("KERNEL_GBUFS", "3"))
MASKGP = bool(int(os.environ.get("KERNEL_MASKGP", "0")))
SPLITQ = bool(int(os.environ.get("KERNEL_SPLITQ", "1")))
VCHUNK = bool(int(os.environ.get("KERNEL_VCHUNK", "0")))
# bf16 coupling path removed: it triggered NRT_EXEC_UNIT_UNRECOVERABLE on HW
BF16C = False
ALS_INPUT = True
NSUB = int(os.environ.get("KERNEL_NSUB", "4"))
HALF = bool(int(os.environ.get("KERNEL_HALF", "0")))

_cache: dict = {}


def _build(sweeps: int, nsub: int, half: bool, t_steps: int = T):
    BB = nsub * B
    nsb = t_steps // BB
    W = nsub * MROWS
    nc = bacc.Bacc(None, target_bir_lowering=False)

    xt = nc.dram_tensor("xt", [N_IN, t_steps], F32, kind="ExternalInput")
    xtr = nc.dram_tensor("xtr", [N_IN, t_steps], F32R, kind="ExternalInput")
    xb_dt = BF16 if BF16C else F32
    xb = nc.dram_tensor("xb", [t_steps, N_IN], xb_dt, kind="ExternalInput")
    v0t = nc.dram_tensor("v0t", [N_IN, MROWS], F32, kind="ExternalInput")
    al = nc.dram_tensor("al", [B, B], F32R, kind="ExternalInput")     # a*(s<=tau)
    als = nc.dram_tensor("als", [B, B], F32R, kind="ExternalInput")   # a*(s<tau)
    aones = nc.dram_tensor("aones", [B, B], F32R, kind="ExternalInput")  # a full
    maskrow = nc.dram_tensor("maskrow", [B, 4 * B], F32, kind="ExternalInput")
    ones1 = nc.dram_tensor("ones1", [1, B], F32R, kind="ExternalInput")
    aonescol = nc.dram_tensor("aonescol", [B, 2], F32R, kind="ExternalInput")
    one11 = nc.dram_tensor("one11", [1, 1], F32, kind="ExternalInput")
    qqz = nc.dram_tensor("qqz", [1, 4 * MROWS], F32R, kind="ExternalInput")
    yout = nc.dram_tensor("yout", [t_steps, MROWS], F32, kind="ExternalOutput")

    with tile.TileContext(nc) as tc:
        with (
            tc.tile_pool(name="const", bufs=1) as cpool,
            tc.tile_pool(name="state", bufs=1) as spool,
            tc.tile_pool(name="xall", bufs=1) as xap,
            tc.tile_pool(name="gh", bufs=GBUFS) as ghp,
            tc.tile_pool(name="sw", bufs=3) as swp,
            tc.tile_pool(name="ps", bufs=1, space="PSUM") as psp,
            tc.tile_pool(name="psg", bufs=GBUFS, space="PSUM") as psgp,
        ):
            # constants
            al_t = cpool.tile([B, B], F32R, tag="al")
            als_t = cpool.tile([B, B], F32R, tag="als")
            aones_t = cpool.tile([B, B], F32R, tag="aones")
            maskrow_t = cpool.tile([B, 4 * B], F32, tag="maskrow")
            ones1_t = cpool.tile([1, B], F32R, tag="ones1")
            aonescol_t = cpool.tile([B, 2], F32R, tag="aonescol")
            one11_t = cpool.tile([1, 1], F32, tag="one11")
            const_dmas = ((al_t, al), (als_t, als), (aones_t, aones),
                          (maskrow_t, maskrow), (ones1_t, ones1),
                          (aonescol_t, aonescol), (one11_t, one11))

            # persistent state (ping-pong) -- DMA'd BEFORE the X preload so
            # block 0's Z0/sigmoid are not head-of-line blocked by ~45us of X
            vhat = [spool.tile([128, 4 * MROWS], F32, tag=f"vhat{p}", name=f"vhat{p}")
                    for p in range(2)]
            qq = [spool.tile([1, W], F32R, tag=f"qq{p}", name=f"qq{p}")
                  for p in range(2)]
            for c in range(4):
                nc.sync.dma_start(
                    vhat[0][:, c * MROWS:(c + 1) * MROWS],
                    v0t[128 * c:128 * (c + 1), :],
                )
            nc.sync.dma_start(qq[0][:], qqz[0:1, 0:W])

            # X resident in SBUF, one tile pair per super-block. Block 0 is
            # loaded via HWDGE (fast start); the rest stream in on the SWDGE
            # queues so they do not head-of-line block anything.
            xt_tiles, xb_tiles = [], []
            for k in range(nsb):
                t0 = k * BB
                eng = nc.sync if k == 0 else nc.gpsimd
                eng2 = (nc.gpsimd if SPLITQ else nc.sync) if k == 0 else nc.gpsimd
                xt_k = xap.tile([128, 4 * BB], F32, tag=f"xtk{k}", name=f"xtk{k}")
                for c in range(4):
                    eng2.dma_start(
                        xt_k[:, c * BB:(c + 1) * BB],
                        xt[128 * c:128 * (c + 1), t0:t0 + BB],
                    )
                xtr_k = xap.tile([128, 4 * BB], F32R, tag=f"xtrk{k}", name=f"xtrk{k}")
                for c in range(4):
                    eng.dma_start(
                        xtr_k[:, c * BB:(c + 1) * BB],
                        xtr[128 * c:128 * (c + 1), t0:t0 + BB],
                    )
                xb_k = xap.tile([128, nsub * N_IN], xb_dt, tag=f"xbk{k}", name=f"xbk{k}")
                for j in range(nsub):
                    eng2.dma_start(
                        xb_k[:, j * N_IN:(j + 1) * N_IN],
                        xb[t0 + j * B:t0 + (j + 1) * B, :],
                    )
                xt_tiles.append((xt_k, xtr_k))
                xb_tiles.append(xb_k)
                if k == 0:
                    # constants ride HWDGE right after block 0's X: they are
                    # needed slightly later than the first Gram matmuls
                    for tl, dr in const_dmas:
                        nc.sync.dma_start(tl[:], dr[:])

            def sub(t_, j):  # column group j of a [128, W] sweep tile
                return t_[:, j * MROWS:(j + 1) * MROWS]

            for k in range(nsb):
                cur, nxt = k % 2, (k + 1) % 2
                t0 = k * BB
                (xt_t, xtr_t), xb_t = xt_tiles[k], xb_tiles[k]

                def xts(c, j):  # xt chunk c, sub-tile j  [128, 128]
                    return xt_t[:, c * BB + j * B:c * BB + (j + 1) * B]

                def xtrs(c, j):  # rounded-f32r xt chunk c, sub-tile j
                    return xtr_t[:, c * BB + j * B:c * BB + (j + 1) * B]

                def xtrsw(c, src):  # rounded xt chunk c, sub-tiles src..
                    return xtr_t[:, c * BB + src * B:(c + 1) * BB]

                def xbs(j, c):  # xb rows of sub j, j-chunk c  [128, 128]
                    return xb_t[:, j * N_IN + c * B:j * N_IN + (c + 1) * B]

                # ---- Gram, batched per src: psg = [G(src,src) .. G(src,nsub-1)] ----
                ghat = []  # ghat[src] = [128, (nsub-src)*B], col g*B+ = dst=src+g
                for src in range(nsub):
                    wn = (nsub - src) * B
                    psg = psgp.tile([128, 4 * B], F32, tag="g", name="psg")
                    for c in range(4):
                        nc.tensor.matmul(
                            psg[:, :wn], xtrs(c, src), xtrsw(c, src),
                            start=(c == 0), stop=(c == 3),
                        )
                    gt = ghp.tile([128, 4 * B], BF16 if BF16C else F32,
                                  tag=f"gh{src}", name=f"gh{src}")
                    eng_g = nc.gpsimd if MASKGP else nc.vector
                    eng_g.tensor_tensor(gt[:, :wn], psg[:, :wn],
                                        maskrow_t[:, :wn], ALU.mult)
                    ghat.append(gt)

                def gslice(src, dst):
                    g = dst - src
                    return ghat[src][:, g * B:(g + 1) * B]

                # ---- Z0 = X_blk @ V^T ----
                psz = psp.tile([128, W], F32, tag="z0")
                for j in range(nsub):
                    for c in range(4):
                        nc.tensor.matmul(
                            sub(psz, j), xts(c, j),
                            vhat[cur][:, c * MROWS:(c + 1) * MROWS],
                            start=(c == 0), stop=(c == 3),
                        )
                z0sb = swp.tile([128, W], F32, tag="z0sb")
                if Z0ACT:
                    nc.scalar.copy(z0sb[:], psz[:])
                else:
                    nc.vector.tensor_copy(z0sb[:], psz[:])
                ycur = swp.tile([128, W], F32, tag="Y")
                nc.scalar.activation(ycur[:], psz[:], ACTF.Sigmoid)

                def s_psum(y2t, strict=False):
                    """S (incl or excl) for all sub-tiles as one [128, W] psum."""
                    ps = psp.tile([128, W], F32, tag="sex" if strict else "sin",
                                  name="ps_s")
                    tri = als_t if strict else al_t
                    nc.tensor.matmul(ps[:], tri[:], y2t[:], start=True, stop=False)
                    for jp in range(nsub - 1):
                        for j in range(jp + 1, nsub):
                            nc.tensor.matmul(sub(ps, j), aones_t[:], sub(y2t, jp),
                                             start=False, stop=False)
                    nc.tensor.matmul(ps[:], ones1_t[:], qq[cur][:],
                                     start=False, stop=True)
                    return ps
                s_incl = s_psum

                uh = None
                y2 = None
                def make_y2(ytile):
                    y2 = swp.tile([128, W], F32R, tag="y2", name="y2")
                    if Y2ACT:
                        nc.scalar.activation(y2[:], ytile[:], ACTF.Square)
                    else:
                        nc.vector.tensor_tensor(y2[:], ytile[:], ytile[:], ALU.mult)
                    return y2

                for r in range(sweeps):
                    y2 = make_y2(ycur)
                    psin = s_incl(y2)
                    psex = s_psum(y2, strict=True) if EEXPE else None
                    # uh' = (S_incl - 1) * Y   (negated u-hat)
                    uh = swp.tile([128, W], BF16 if BF16C else F32, tag="uh")
                    nc.vector.scalar_tensor_tensor(
                        uh[:], psin[:], 1.0, ycur[:], ALU.subtract, ALU.mult)
                    if not EEXPE:
                        # E_ex = 1 + S_ex = (S_in + 1) - a*y^2
                        ly = swp.tile([128, W], F32, tag="ly")
                        nc.vector.tensor_scalar(ly[:], _f(y2[:]),
                                                _f(aonescol_t[:, 0:1]),
                                                None, ALU.mult)
                        eex = swp.tile([128, W], F32, tag="eex")
                        nc.vector.scalar_tensor_tensor(
                            eex[:], psin[:], 1.0, ly[:], ALU.add, ALU.subtract)
                    psc = psp.tile([128, W], F32, tag="c")
                    for dst in range(nsub):
                        for src in range(dst + 1):
                            nc.tensor.matmul(
                                sub(psc, dst), gslice(src, dst), sub(uh, src),
                                start=(src == 0), stop=(src == dst),
                            )
                    zpre = swp.tile([128, W], F32, tag="zpre")
                    nc.vector.scalar_tensor_tensor(
                        zpre[:], psc[:], -1.0, z0sb[:], ALU.mult, ALU.add)
                    zt = swp.tile([128, W], F32, tag="z")
                    if EEXPE:
                        nc.vector.scalar_tensor_tensor(
                            zt[:], psex[:], 1.0, zpre[:], ALU.add, ALU.mult)
                    else:
                        nc.vector.tensor_tensor(zt[:], eex[:], zpre[:], ALU.mult)
                    ynew = swp.tile([128, W], F32, tag="Y")
                    nc.scalar.activation(ynew[:], zt[:], ACTF.Sigmoid)
                    ycur = ynew

                if half:
                    y2 = make_y2(ycur)
                    psin = s_incl(y2)
                    uh = swp.tile([128, W], BF16 if BF16C else F32, tag="uh")
                    nc.vector.scalar_tensor_tensor(
                        uh[:], psin[:], 1.0, ycur[:], ALU.subtract, ALU.mult)

                # ---- QQ_next = QQ + a*colsum(y2) ----
                psqq = psp.tile([2, MROWS], F32, tag="qq", name="psqq")
                for j in range(nsub):
                    nc.tensor.matmul(psqq[:], aonescol_t[:], sub(y2, j),
                                     start=(j == 0), stop=(j == nsub - 1))
                for j in range(nsub):
                    nc.vector.tensor_tensor(sub(qq[nxt], j), psqq[0:1, :],
                                            _f(qq[cur][:, 0:MROWS]), ALU.add)

                # ---- V update: vhat_next = vhat_cur - Xb_blk^T @ uh' ----
                psv = psp.tile([128, 4 * MROWS], F32, tag="dv")
                for c in range(4):
                    for j in range(nsub):
                        nc.tensor.matmul(
                            psv[:, c * MROWS:(c + 1) * MROWS],
                            xbs(j, c), sub(uh, j),
                            start=(j == 0), stop=(j == nsub - 1),
                        )
                    if VCHUNK:
                        nc.vector.tensor_tensor(
                            vhat[nxt][:, c * MROWS:(c + 1) * MROWS],
                            vhat[cur][:, c * MROWS:(c + 1) * MROWS],
                            psv[:, c * MROWS:(c + 1) * MROWS], ALU.subtract)
                if not VCHUNK:
                    nc.vector.tensor_tensor(vhat[nxt][:], vhat[cur][:], psv[:],
                                            ALU.subtract)

                for j in range(nsub):
                    nc.sync.dma_start(
                        yout[t0 + j * B:t0 + (j + 1) * B, :], sub(ycur, j)
                    )

    nc.compile()
    return nc


def _get_nc(sweeps: int = SWEEPS, nsub: int = NSUB, half: bool = HALF,
            t_steps: int = T):
    key = (sweeps, nsub, half, t_steps, Y2ACT, Z0ACT, EEXPE, GBUFS, MASKGP, SPLITQ, VCHUNK, BF16C)
    if key not in _cache:
        _cache[key] = _build(sweeps, nsub, half, t_steps)
    return _cache[key]


def _round_f32r(x):
    """Round fp32 to f32r (11-bit mantissa, RNE) -- bits stay valid fp32."""
    b_ = np.ascontiguousarray(x, dtype=np.float32).view(np.uint32)
    lsb = (b_ >> 12) & 1
    out = (b_ + 0x7FF + lsb) & np.uint32(0xFFFFF000)
    return out.view(np.float32)


def _host_inputs(X, W_init, a, b):
    ones = np.ones((B, B), np.float32)
    maskrow = np.concatenate(
        [b * np.triu(ones, 1)] + [np.full((B, B), b, np.float32)] * 3, axis=1)
    common = {
        "xt": np.ascontiguousarray(X.T),
        "xtr": _round_f32r(X.T),
        # xb dtype must match the kernel build (bf16 when KERNEL_BF16C=1)

        "xb": (np.ascontiguousarray(b * X) if not BF16C else
               np.ascontiguousarray(b * X).astype(mybir.dt.np(BF16))),
        "al": _round_f32r(a * np.triu(ones)),
        "als": _round_f32r(a * np.triu(ones, 1)),
        "aones": _round_f32r(np.full((B, B), a, np.float32)),
        "maskrow": maskrow.astype(np.float32),
        "ones1": np.ones((1, B), np.float32),
        "qqz": np.zeros((1, 4 * MROWS), np.float32),
        "aonescol": _round_f32r(
            np.concatenate([np.full((B, 1), a, np.float32),
                            np.zeros((B, 1), np.float32)], axis=1)),
        "one11": np.ones((1, 1), np.float32),
    }
    maps = []
    for c in range(NCORES):
        m = dict(common)
        m["v0t"] = np.ascontiguousarray(W_init[c * MROWS:(c + 1) * MROWS, :].T)
        maps.append(m)
    return maps


last_results = None


def kernel(X, W_init, theta, observed_idx):
    global last_results
    X = np.asarray(X, dtype=np.float32)
    W_init = np.asarray(W_init, dtype=np.float32)
    theta = np.asarray(theta, dtype=np.float32)
    observed_idx = np.asarray(observed_idx)

    lr = 1.0 / N_IN
    a = lr * float(theta[1])
    b = lr * float(theta[0])
    assert abs(a) * T < 0.02, f"|a|*T={abs(a)*T:.3e} too large for linear exp"

    nc = _get_nc()
    in_maps = _host_inputs(X, W_init, a, b)
    trace = bool(int(os.environ.get("KERNEL_TRACE", "0")))
    res = run_bass_kernel_spmd(nc, in_maps, list(range(NCORES)), trace=trace)
    last_results = res
    y_full = np.concatenate(
        [res.results[c]["yout"] for c in range(NCORES)], axis=1
    )
    return np.ascontiguousarray(y_full[:, observed_idx]).astype(np.float32)
